# revision 1
# baseline (speedup 1.0000x reference)
"""Trainium2 Bass kernel for a character-CNN word encoder.

Computation (per word of W=20 chars):
  x = emb[chars]                       # [W, E=64] -> [E, W]
  y_k = conv1d(x, w_k, 'same') + b_k   # k in {1,3,5}, H=256 channels
  m_k = max_t relu(y_k)                # [H]
  out = concat(m1, m3, m5) @ lw.T + lb # [H]

Strategy (pure data parallel over N = B*S = 8192 words, 1024 words/core):
  - Embedding gather via dma_gather(transpose=True) from a [257, 128] bf16
    table (row 256 = zeros = padding token, cols 64..127 = zeros). The
    index stream inserts 2 pad tokens on each side of every word, so the
    gather directly materializes X in SBUF as [128 part (E padded),
    words * 24] with 'same'-conv zero padding built in. Indices are int16,
    wrapped into 16 partitions and replicated to all 128; single_packet
    must be False at this descriptor count (True crashes the device).
  - A SBUF->SBUF DMA writes rows 64:128 with X shifted left by one column,
    so a single K=128 matmul contracts TWO consecutive conv taps at once.
  - Convs = tap-pair matmuls accumulating in PSUM: per 16-word block and
    H-half, conv5 needs 3 matmuls, conv3 2, conv1 1 (N = 320 columns each).
  - Max over positions: DVE tensor_tensor(max) PSUM halves -> bf16, then
    tensor_reduce(max); one 3-region PSUM group goes through ACT copy
    instead to balance engines. relu(bias + max) on ACT afterwards.
  - Linear layer: 6 K=128 matmuls (m as stationary) + a K=1 ones-row matmul
    that adds lb. Output lands as [words, 256] fp32, DMAed contiguously.
"""

import numpy as np
import ml_dtypes

import concourse.bass as bass
import concourse.tile as tile
import concourse.mybir as mybir
from concourse import bacc
from concourse.bass_utils import run_bass_kernel_spmd

BF16 = ml_dtypes.bfloat16

# Problem shape (hardcoded per contest rules).
B, S, W = 64, 128, 20
VOCAB, E, H = 256, 64, 256
N_CORES = 8
NW = (B * S) // N_CORES       # words per core = 1024
WP = 22                       # word frame: [z z t0..t19]; right pads are the
                              # NEXT word's left pads (halo zeros at chunk end)
PAD_TOK = VOCAB               # index of the all-zero table row
CHUNK_W = 128                 # words per gather chunk
N_CHUNKS = NW // CHUNK_W      # 8
NB = 16                       # words per matmul block
N_BLOCKS = CHUNK_W // NB      # 8
IDX_PER_CHUNK = CHUNK_W * WP      # 2816 tokens per chunk
IDX_COLS = NW * WP // 16          # 1408 (token t at idx[t % 16, t // 16])

# Conv tap-pair matmul plan.
# Each entry: (region, conv_k, half, first_tap d, j0 = d - pad + 2, start, stop)
# Regions: 0=c5h0 1=c5h1 2=c3h0 3=c3h1 4=c1h0 5=c1h1.
# Group A (PSUM tile A, regions 0..2, processed by DVE max path),
# group B (PSUM tile B, regions 3..5, processed by ACT copy path).
def _mm_plan():
    plan_a, plan_b = [], []
    for half in (0, 1):
        r = half  # conv5 regions 0,1
        taps = [(0, 0), (2, 2), (4, 4)]  # (d, j0) for pad=2
        for i, (d, j0) in enumerate(taps):
            plan_a.append((r, 5, half, d, j0, i == 0, i == len(taps) - 1))
    # conv3 h0 -> region 2 (tile A); conv3 h1 -> region 3 (tile B)
    for half, r, dst in ((0, 2, plan_a), (1, 3, plan_b)):
        taps = [(0, 1), (2, 3)]  # pad=1
        for i, (d, j0) in enumerate(taps):
            dst.append((r, 3, half, d, j0, i == 0, i == len(taps) - 1))
    for half in (0, 1):
        r = 4 + half  # conv1 regions 4,5
        plan_b.append((r, 1, half, 0, 2, True, True))
    return plan_a, plan_b

PLAN_A, PLAN_B = _mm_plan()
# Weight block order: all of plan A then plan B -> 12 blocks of [128, 128].
WBLOCKS = PLAN_A + PLAN_B

# lw column ranges per region (reference concat order: conv1, conv3, conv5).
LW_COLS = {0: (512, 640), 1: (640, 768), 2: (256, 384), 3: (384, 512),
           4: (0, 128), 5: (128, 256)}


def _build_nc():
    f32 = mybir.dt.float32
    bf16 = mybir.dt.bfloat16
    i16 = mybir.dt.int16
    AF = mybir.ActivationFunctionType
    ALU = mybir.AluOpType
    AX = mybir.AxisListType

    nc = bacc.Bacc("TRN2", target_bir_lowering=False, debug=False)

    idx_d = nc.dram_tensor("idx", [128, IDX_COLS], i16, kind="ExternalInput").ap()
    table_d = nc.dram_tensor("table", [VOCAB + 1, 128], bf16, kind="ExternalInput").ap()
    wconv_d = nc.dram_tensor("wconv", [128, 12 * 128], bf16, kind="ExternalInput").ap()
    lwt_d = nc.dram_tensor("lwt", [128, 6 * 256], bf16, kind="ExternalInput").ap()
    cbias_d = nc.dram_tensor("cbias", [128, 6], f32, kind="ExternalInput").ap()
    lbias_d = nc.dram_tensor("lbias", [1, 256], bf16, kind="ExternalInput").ap()
    out_d = nc.dram_tensor("out", [NW, H], f32, kind="ExternalOutput").ap()

    with tile.TileContext(nc) as tc:
        with (
            tc.tile_pool(name="consts", bufs=1) as cpool,
            tc.tile_pool(name="xx", bufs=4) as xxpool,
            tc.tile_pool(name="ybuf", bufs=3) as ybpool,
            tc.tile_pool(name="mtile", bufs=2) as mpool,
            tc.tile_pool(name="osb", bufs=2) as opool,
            tc.tile_pool(name="psA", bufs=2, space="PSUM") as psa_pool,
            tc.tile_pool(name="psO", bufs=2, space="PSUM") as pso_pool,
        ):
            # --- constants (idx first: it alone gates the gathers) ---
            idx_sb = cpool.tile([128, IDX_COLS], i16, tag="idx")
            nc.sync.dma_start(idx_sb[:], idx_d[:])
            wconv_sb = cpool.tile([128, 12 * 128], bf16, tag="wconv")
            nc.sync.dma_start(wconv_sb[:], wconv_d[:])
            lwt_sb = cpool.tile([128, 6 * 256], bf16, tag="lwt")
            nc.sync.dma_start(lwt_sb[:], lwt_d[:])
            cbias_sb = cpool.tile([128, 6], f32, tag="cbias")
            nc.sync.dma_start(cbias_sb[:], cbias_d[:])
            lbias_sb = cpool.tile([1, 256], bf16, tag="lbias")
            nc.sync.dma_start(lbias_sb[:], lbias_d[:])
            ones_sb = cpool.tile([1, 128], bf16, tag="ones")
            nc.vector.memset(ones_sb[:], 1.0)

            # --- gather pipeline: queue all chunks up front so the Q7
            # descriptor generator (the serial bottleneck) never idles ---
            xx_tiles = []
            for c in range(N_CHUNKS):
                xx = xxpool.tile([128, IDX_PER_CHUNK + 8], bf16, tag="xx")
                nc.gpsimd.dma_gather(
                    xx[:, 0:IDX_PER_CHUNK].rearrange("p (a n) -> p a n", a=1),
                    table_d[:],
                    idx_sb[:, c * (IDX_PER_CHUNK // 16):(c + 1) * (IDX_PER_CHUNK // 16)],
                    num_idxs=IDX_PER_CHUNK,
                    num_idxs_reg=IDX_PER_CHUNK,
                    elem_size=128,
                    transpose=True,
                    single_packet=False,
                )
                nc.vector.memset(xx[:, IDX_PER_CHUNK:], 0.0)
                # rows 64:128 = rows 0:64 shifted left one column
                nc.sync.dma_start(
                    xx[64:128, 0:IDX_PER_CHUNK], xx[0:64, 1:IDX_PER_CHUNK + 1]
                )
                xx_tiles.append(xx)

            for c in range(N_CHUNKS):
                xx = xx_tiles[c]
                m_pre = mpool.tile([128, 6, CHUNK_W], bf16, tag="m_pre")
                m_all = mpool.tile([128, 6, CHUNK_W], bf16, tag="m_all")

                for b in range(N_BLOCKS):
                    base = b * NB * WP

                    def run_mms(plan, ps):
                        for (r, _k, _h, _d, j0, start, stop) in plan:
                            q = WBLOCKS.index((r, _k, _h, _d, j0, start, stop))
                            slot = r % 3
                            rhs = (
                                xx[:, base + j0: base + j0 + NB * WP]
                                .rearrange("p (w c) -> p w c", c=WP)[:, :, 0:W]
                            )
                            nc.tensor.matmul(
                                ps[:, slot * 512: slot * 512 + NB * W],
                                lhsT=wconv_sb[:, q * 128:(q + 1) * 128],
                                rhs=rhs,
                                start=start,
                                stop=stop,
                            )

                    ps_a = psa_pool.tile([128, 1536], f32, tag="psA")
                    run_mms(PLAN_A, ps_a)
                    ps_b = psa_pool.tile([128, 1536], f32, tag="psA")
                    run_mms(PLAN_B, ps_b)

                    # DVE path for tile A: reduce-max straight from PSUM
                    # (tensor_tensor with two PSUM operands is illegal on HW)
                    pa = (
                        ps_a[:, 0:1536]
                        .rearrange("p (r b) -> p r b", b=512)[:, :, 0:NB * W]
                        .rearrange("p r (w c) -> p r w c", c=W)
                    )
                    nc.vector.tensor_reduce(
                        out=m_pre[:, 0:3, b * NB:(b + 1) * NB],
                        in_=pa, axis=AX.X, op=ALU.max,
                    )

                    # ACT path for tile B: copy PSUM -> bf16 SBUF, DVE reduce
                    pb = (
                        ps_b[:, 0:1536]
                        .rearrange("p (r b) -> p r b", b=512)[:, :, 0:NB * W]
                        .rearrange("p r (w c) -> p r w c", c=W)
                    )
                    yb = ybpool.tile([128, 3 * NB * W], bf16, tag="yb")
                    ybv = yb[:].rearrange("p (r w c) -> p r w c", r=3, w=NB)
                    nc.scalar.copy(out=ybv, in_=pb)
                    nc.vector.tensor_reduce(
                        out=m_pre[:, 3:6, b * NB:(b + 1) * NB],
                        in_=ybv, axis=AX.X, op=ALU.max,
                    )

                # bias + relu per region
                for r in range(6):
                    nc.scalar.activation(
                        m_all[:, r, :], m_pre[:, r, :], AF.Relu,
                        bias=cbias_sb[:, r:r + 1],
                    )

                # linear layer: out[words, 256] = m.T @ lwT + lb
                op = pso_pool.tile([128, 256], f32, tag="psO")
                for r in range(6):
                    nc.tensor.matmul(
                        op[:], lhsT=m_all[:, r, :], rhs=lwt_sb[:, r * 256:(r + 1) * 256],
                        start=(r == 0), stop=False,
                    )
                nc.tensor.matmul(
                    op[:], lhsT=ones_sb[0:1, :], rhs=lbias_sb[0:1, :],
                    start=False, stop=True,
                )
                osb = opool.tile([128, 256], f32, tag="osb")
                nc.scalar.copy(out=osb[:], in_=op[:])
                nc.sync.dma_start(out_d[c * CHUNK_W:(c + 1) * CHUNK_W, :], osb[:])

    nc.compile()
    return nc


def _prep_maps(chars, emb, w1, b1, w3, b3, w5, b5, lw, lb):
    flat = np.asarray(chars).reshape(-1, W).astype(np.int64)  # [8192, 20]
    emb = np.asarray(emb, dtype=np.float32)
    lw = np.asarray(lw, dtype=np.float32)
    convs = {1: np.asarray(w1, np.float32), 3: np.asarray(w3, np.float32),
             5: np.asarray(w5, np.float32)}
    biases = {1: np.asarray(b1, np.float32), 3: np.asarray(b3, np.float32),
              5: np.asarray(b5, np.float32)}

    table = np.zeros((VOCAB + 1, 128), dtype=BF16)
    table[:VOCAB, :E] = emb.astype(BF16)

    wconv = np.zeros((128, 12 * 128), dtype=BF16)
    for q, (r, k, half, d, j0, _s, _e) in enumerate(WBLOCKS):
        wk = convs[k]  # [H, E, k]
        blk = np.zeros((128, 128), dtype=np.float32)
        blk[:E, :] = wk[half * 128:(half + 1) * 128, :, d].T
        if d + 1 < k:
            blk[E:, :] = wk[half * 128:(half + 1) * 128, :, d + 1].T
        wconv[:, q * 128:(q + 1) * 128] = blk.astype(BF16)

    lwt = np.zeros((128, 6 * 256), dtype=BF16)
    for r in range(6):
        lo, hi = LW_COLS[r]
        lwt[:, r * 256:(r + 1) * 256] = lw[:, lo:hi].T.astype(BF16)

    cbias = np.zeros((128, 6), dtype=np.float32)
    for r, (k, half) in enumerate([(5, 0), (5, 1), (3, 0), (3, 1), (1, 0), (1, 1)]):
        cbias[:, r] = biases[k][half * 128:(half + 1) * 128]

    lbias = np.asarray(lb, np.float32).reshape(1, 256).astype(BF16)

    in_maps = []
    for c in range(N_CORES):
        words = flat[c * NW:(c + 1) * NW]  # [NW, 20]
        padded = np.full((NW, WP), PAD_TOK, dtype=np.int16)
        padded[:, 2:2 + W] = words
        stream = padded.reshape(-1)  # [NW*24]
        wrapped = stream.reshape(-1, 16).T  # [16, IDX_COLS]
        idx = np.ascontiguousarray(np.tile(wrapped, (8, 1)))  # replicated x8
        in_maps.append({
            "idx": idx, "table": table, "wconv": wconv, "lwt": lwt,
            "cbias": cbias, "lbias": lbias,
        })
    return in_maps


_NC_CACHE = {}


def run(inputs, trace=False):
    if "nc" not in _NC_CACHE:
        _NC_CACHE["nc"] = _build_nc()
    nc = _NC_CACHE["nc"]
    in_maps = _prep_maps(**inputs)
    res = run_bass_kernel_spmd(nc, in_maps, list(range(N_CORES)), trace=trace)
    out = np.concatenate([res.results[i]["out"] for i in range(N_CORES)], axis=0)
    return out.reshape(B, S, H).astype(np.float32), res


def kernel(**inputs):
    out, _ = run(inputs)
    return out

